# revision 1
# baseline (speedup 1.0000x reference)
"""Distributed Trainium2 (Bass/Tile) kernel for the DQN-style GNN message-passing
module.

Full-input contract: ``kernel(**inputs)`` takes the unsharded inputs exactly as
produced by ``setup_inputs()`` and returns the full output (shape ``(1,)``).

Sharding strategy (1D node partition, per the hint):
  - graph [N, N] row-sharded across 8 cores -> [N/8, N] per core
  - x / u row-sharded; thetas replicated
  - per-iteration global sum u.sum(0) -> AllReduce of a [dim, 1] vector
  - final readout: AllReduce of [dim, 2] carrying [sum(u) | one-hot-masked u[v]]

Math (mirrors the reference's exact relu identity):
  c[r, :] = s_abs[r] * A + s_sum[r] * B
    with s_abs[r] = sum_j |g[r, j]|, s_sum[r] = sum_j g[r, j],
         A = 0.5 * |theta4| @ theta3, B = 0.5 * theta4 @ theta3
  a[r, :] = xf[r] * theta1

  Per-core state tile US [64 + 1 + 2*NCH, R] stacks u^T over per-row scalars
  [xf; abs chunk sums; plain chunk sums].  With the host-built stationary
  M = [-theta2; theta1; A (x NCH); B (x NCH)], a single matmul computes
    pre^T = M^T @ US = (a + c - u @ theta2)^T
  and the iteration is  u' = ReluAct(pre, bias = (S_t @ theta2)^T)  where
  S_t comes from a [64,1] AllReduce of the local row-sum of u.

  (Splitting s_abs/s_sum by column chunk and duplicating the A/B rows lets the
  matmul absorb the chunk-combine add for free.)
"""

from contextlib import ExitStack

import numpy as np

import concourse.bass as bass
import concourse.tile as tile
from concourse import bacc, mybir
from concourse.bass_utils import run_bass_kernel_spmd

F32 = mybir.dt.float32
BF16 = mybir.dt.bfloat16
AX = mybir.AxisListType
ALU = mybir.AluOpType
ACTF = mybir.ActivationFunctionType

N_CORES = 8
DIM = 64

_program_cache: dict = {}


def _chunking(C: int):
    CH = 4096 if C >= 4096 else C
    return CH, C // CH


def build_program(R: int, C: int, D: int, T: int, n_cores: int = N_CORES):
    """Build + compile the per-core SPMD Bass program.

    R: local rows (N / n_cores), C: row length (N), D: dim, T: iterations.
    """
    assert R % 128 == 0 and D == 64
    NB = R // 128
    CH, NCH = _chunking(C)
    KM = D + 1 + 2 * NCH  # stationary contraction size
    FB = 512  # matmul free-dim chunk (one PSUM bank of f32)

    nc = bacc.Bacc(
        "TRN2",
        target_bir_lowering=False,
        debug=False,
        enable_asserts=True,
        num_devices=n_cores,
    )
    g_d = nc.dram_tensor("g", [R, C], F32, kind="ExternalInput")
    xf_d = nc.dram_tensor("xf", [1, R], F32, kind="ExternalInput")
    vsel_d = nc.dram_tensor("vsel", [1, R], F32, kind="ExternalInput")
    mneg_d = nc.dram_tensor("mneg", [KM, D], F32, kind="ExternalInput")
    t2_d = nc.dram_tensor("t2", [D, D], F32, kind="ExternalInput")
    t6_d = nc.dram_tensor("t6", [D, D], F32, kind="ExternalInput")
    t7_d = nc.dram_tensor("t7", [D, D], F32, kind="ExternalInput")
    t5c_d = nc.dram_tensor("t5c", [2 * D, 1], F32, kind="ExternalInput")
    out_d = nc.dram_tensor("out", [1, 1], F32, kind="ExternalOutput")
    ident_d = nc.inline_tensor(np.eye(128, dtype=np.float32), name="ident")

    rg = [list(range(n_cores))]

    with ExitStack() as ctx:
        tc = ctx.enter_context(tile.TileContext(nc))
        const = ctx.enter_context(tc.tile_pool(name="const", bufs=1))
        small = ctx.enter_context(tc.tile_pool(name="small", bufs=1))
        gp = ctx.enter_context(tc.tile_pool(name="gp", bufs=4))
        scr = ctx.enter_context(tc.tile_pool(name="scr", bufs=2))
        tbsp = ctx.enter_context(tc.tile_pool(name="tbs", bufs=2))
        slp = ctx.enter_context(tc.tile_pool(name="sl", bufs=2))
        dram = ctx.enter_context(tc.tile_pool(name="dram", bufs=2, space="DRAM"))

        # ---- warm-up collective (absorbs first-CC init under phase 1's DMA)
        dwi = dram.tile([D, 1], F32, tag="dwi")
        dwo = dram.tile([D, 1], F32, tag="dwo")
        nc.gpsimd.collective_compute(
            "AllReduce",
            ALU.add,
            replica_groups=rg,
            ins=[dwi[:].opt()],
            outs=[dwo[:].opt()],
        )

        # ---- constants / persistent tiles
        ident = const.tile([128, 128], F32)
        nc.scalar.dma_start(out=ident[:], in_=ident_d.ap())
        mneg = const.tile([KM, D], F32)
        nc.scalar.dma_start(out=mneg[:], in_=mneg_d.ap())
        t2 = const.tile([D, D], F32)
        nc.scalar.dma_start(out=t2[:], in_=t2_d.ap())
        t6 = const.tile([D, D], F32)
        nc.scalar.dma_start(out=t6[:], in_=t6_d.ap())
        t7 = const.tile([D, D], F32)
        nc.scalar.dma_start(out=t7[:], in_=t7_d.ap())
        t5c = const.tile([2 * D, 1], F32)
        nc.scalar.dma_start(out=t5c[:], in_=t5c_d.ap())
        vsel = const.tile([1, R], F32)
        nc.scalar.dma_start(out=vsel[:], in_=vsel_d.ap())
        ones1d = const.tile([1, D], F32)
        nc.vector.memset(ones1d[:], 1.0)

        # state tiles: [u (64 rows); xf; abs sums; plain sums]
        USa = small.tile([KM, R], F32)
        USb = small.tile([KM, R], F32)
        for us in (USa, USb):
            nc.scalar.dma_start(out=us[D : D + 1, :], in_=xf_d.ap())

        # vsel replicated across the 64 partitions (for the final masked row)
        vrepS = small.tile([D, R], F32)
        with tc.tile_pool(name="psV", bufs=1, space="PSUM") as psV:
            vrep = psV.tile([D, R], F32, tag="vrep")
            for h in range(0, R, FB):
                he = min(R, h + FB)
                nc.tensor.matmul(
                    vrep[:, h:he],
                    lhsT=ones1d[:],
                    rhs=vsel[:, h:he],
                    start=True,
                    stop=True,
                )
            nc.vector.tensor_copy(vrepS[:], vrep[:])

        # ---- phase 1: stream graph; half-batch pipeline down to u1
        # Half-batch hb covers Bh row-blocks; SPA col = j * Bh + b_rel with
        # j in [abs chunk 0.., plain chunk 0..] (2*NCH values).
        HB = 2 if NB % 2 == 0 and NB >= 2 else 1
        Bh = NB // HB
        SPAs = [
            small.tile([128, 2 * NCH * Bh], F32, tag=f"spa{h}", name=f"spa{h}")
            for h in range(HB)
        ]
        slb = small.tile([D, HB], F32)  # per-half partial sums of u1
        with tc.tile_pool(name="psT", bufs=1, space="PSUM") as psT, tc.tile_pool(
            name="psU1", bufs=2, space="PSUM"
        ) as psU1:
            for b in range(NB):
                hb, br = divmod(b, Bh)
                gt = gp.tile([128, C], F32, tag="gt")
                if b == 0 and NCH > 1:
                    half = C // 2
                    nc.sync.dma_start(
                        out=gt[:, 0:half], in_=g_d.ap()[0:128, 0:half]
                    )
                    nc.sync.dma_start(
                        out=gt[:, half:C], in_=g_d.ap()[0:128, half:C]
                    )
                else:
                    nc.sync.dma_start(
                        out=gt[:], in_=g_d.ap()[b * 128 : (b + 1) * 128, :]
                    )
                for k in range(NCH):
                    col = (NCH + k) * Bh + br
                    nc.vector.reduce_sum(
                        out=SPAs[hb][:, col : col + 1],
                        in_=gt[:, k * CH : (k + 1) * CH],
                        axis=AX.X,
                    )
                    st = scr.tile([128, CH], BF16, tag="st")
                    col = k * Bh + br
                    nc.scalar.activation(
                        out=st[:],
                        in_=gt[:, k * CH : (k + 1) * CH],
                        func=ACTF.Abs,
                        accum_out=SPAs[hb][:, col : col + 1],
                    )
                if br == Bh - 1:
                    lo, hi = hb * Bh * 128, (hb + 1) * Bh * 128
                    tb = psT.tile([2 * NCH * Bh, 128], F32, tag="tb")
                    nc.tensor.transpose(
                        out=tb[:], in_=SPAs[hb][:], identity=ident[:]
                    )
                    tbs = tbsp.tile([2 * NCH * Bh, 128], F32, tag="tbs")
                    nc.vector.tensor_copy(tbs[:], tb[:])
                    for us in (USb, USa):
                        nc.scalar.dma_start(out=us[D + 1 : KM, lo:hi], in_=tbs[:])
                    ub = psU1.tile([D, Bh * 128], F32, tag="ub")
                    for h in range(lo, hi, FB):
                        he = min(hi, h + FB)
                        nc.tensor.matmul(
                            ub[:, h - lo : he - lo],
                            lhsT=mneg[D:KM, :],
                            rhs=USb[D:KM, h:he],
                            start=True,
                            stop=True,
                        )
                    nc.scalar.activation(
                        out=USb[0:D, lo:hi],
                        in_=ub[:],
                        func=ACTF.Relu,
                        accum_out=slb[:, hb : hb + 1],
                    )


        # ---- iterations 2..T (alternate state tiles)
        psPRE = ctx.enter_context(tc.tile_pool(name="psPRE", bufs=1, space="PSUM"))
        psZ = ctx.enter_context(tc.tile_pool(name="psZ", bufs=2, space="PSUM"))
        cur, nxt = USb, USa
        SW = small.tile([D, 2], F32)
        sl_cur = slp.tile([D, 1], F32, tag="sl0", name="sl0")
        nc.vector.reduce_sum(out=sl_cur[:], in_=slb[:], axis=AX.X)
        for t in range(1, T):
            cin = dram.tile([D, 1], F32, tag="cin")
            cout = dram.tile([D, 1], F32, tag="cout")
            nc.scalar.dma_start(out=cin[:], in_=sl_cur[:])
            nc.gpsimd.collective_compute(
                "AllReduce",
                ALU.add,
                replica_groups=rg,
                ins=[cin[:].opt()],
                outs=[cout[:].opt()],
            )
            Sg = slp.tile([D, 1], F32, tag="Sg")
            nc.scalar.dma_start(out=Sg[:], in_=cout[:])

            pre = psPRE.tile([D, R], F32, tag="pre")
            for h in range(0, R, FB):
                he = min(R, h + FB)
                nc.tensor.matmul(
                    pre[:, h:he],
                    lhsT=mneg[:],
                    rhs=cur[:, h:he],
                    start=True,
                    stop=True,
                )
            z = psZ.tile([D, 1], F32, tag="z")
            nc.tensor.matmul(z[:], lhsT=t2[:], rhs=Sg[:], start=True, stop=True)
            zs = slp.tile([D, 1], F32, tag="zs")
            nc.scalar.copy(zs[:], z[:])
            if t == T - 1:
                acc = SW[:, 0:1]
            else:
                acc = slp.tile([D, 1], F32, tag="sl", name="sl")
            nc.scalar.activation(
                out=nxt[0:D, :],
                in_=pre[:],
                func=ACTF.Relu,
                bias=zs[:, 0:1],
                accum_out=acc,
            )
            sl_cur = acc
            cur, nxt = nxt, cur

        # ---- final readout
        with tc.tile_pool(name="psF", bufs=1, space="PSUM") as psF:
            u4 = cur[0:D, :]
            scrv = small.tile([D, R], F32)
            nc.vector.scalar_tensor_tensor(
                out=scrv[:],
                in0=u4,
                scalar=1.0,
                in1=vrepS[:],
                op0=ALU.mult,
                op1=ALU.mult,
                accum_out=SW[:, 1:2],
            )
            cin2 = dram.tile([D, 2], F32, tag="cin2")
            cout2 = dram.tile([D, 2], F32, tag="cout2")
            nc.scalar.dma_start(out=cin2[:], in_=SW[:])
            nc.gpsimd.collective_compute(
                "AllReduce",
                ALU.add,
                replica_groups=rg,
                ins=[cin2[:].opt()],
                outs=[cout2[:].opt()],
            )
            SWg = small.tile([D, 2], F32)
            nc.scalar.dma_start(out=SWg[:], in_=cout2[:])

            q = psF.tile([2 * D, 1], F32, tag="q")
            nc.tensor.matmul(
                q[0:D, :], lhsT=t6[:], rhs=SWg[:, 0:1], start=True, stop=True
            )
            nc.tensor.matmul(
                q[D : 2 * D, :], lhsT=t7[:], rhs=SWg[:, 1:2], start=True, stop=True
            )
            rq = small.tile([2 * D, 1], F32)
            nc.scalar.activation(out=rq[:], in_=q[:], func=ACTF.Relu)
            res = psF.tile([1, 1], F32, tag="res")
            nc.tensor.matmul(
                res[:], lhsT=rq[:], rhs=t5c[:], start=True, stop=True
            )
            ress = small.tile([1, 1], F32)
            nc.scalar.copy(ress[:], res[:])
            nc.scalar.dma_start(out=out_d.ap(), in_=ress[:])

    nc.compile()
    return nc


def get_program(R: int, C: int, D: int, T: int, n_cores: int = N_CORES):
    key = (R, C, D, T, n_cores)
    if key not in _program_cache:
        _program_cache[key] = build_program(R, C, D, T, n_cores)
    return _program_cache[key]


def make_in_maps(graph, x, theta1, theta2, theta3, theta4, theta5, theta6, theta7, v,
                 n_cores: int = N_CORES):
    """Host-side sharding + tiny theta preprocessing."""
    N = graph.shape[0]
    D = theta1.shape[1]
    R = N // n_cores
    _, NCH = _chunking(N)
    f32 = np.float32

    t4 = np.asarray(theta4, f32)[0]
    t3 = np.asarray(theta3, f32)
    A = 0.5 * (np.abs(t4) @ t3)
    B = 0.5 * (t4 @ t3)
    t2 = np.ascontiguousarray(np.asarray(theta2, f32))
    mneg = np.ascontiguousarray(
        np.concatenate(
            [-t2, np.asarray(theta1, f32)]
            + [A[None, :]] * NCH
            + [B[None, :]] * NCH,
            axis=0,
        ).astype(f32)
    )  # (D + 1 + 2*NCH, D)
    t5c = np.ascontiguousarray(np.asarray(theta5, f32).reshape(2 * D, 1))
    t6 = np.ascontiguousarray(np.asarray(theta6, f32))
    t7 = np.ascontiguousarray(np.asarray(theta7, f32))

    xf = np.asarray(x).astype(f32)
    vsel_full = np.zeros(N, f32)
    vsel_full[int(v)] = 1.0

    in_maps = []
    for i in range(n_cores):
        sl = slice(i * R, (i + 1) * R)
        in_maps.append(
            {
                "g": np.ascontiguousarray(np.asarray(graph, f32)[sl]),
                "xf": np.ascontiguousarray(xf[sl].reshape(1, R)),
                "vsel": np.ascontiguousarray(vsel_full[sl].reshape(1, R)),
                "mneg": mneg,
                "t2": t2,
                "t6": t6,
                "t7": t7,
                "t5c": t5c,
            }
        )
    return in_maps


def run(inputs: dict, trace: bool = False):
    """Run the distributed kernel on hardware; returns (output, BassKernelResults)."""
    graph = np.asarray(inputs["graph"])
    N = graph.shape[0]
    D = inputs["theta1"].shape[1]
    T = int(inputs["T"])
    R = N // N_CORES

    nc = get_program(R, N, D, T, N_CORES)
    in_maps = make_in_maps(
        graph,
        inputs["x"],
        inputs["theta1"],
        inputs["theta2"],
        inputs["theta3"],
        inputs["theta4"],
        inputs["theta5"],
        inputs["theta6"],
        inputs["theta7"],
        inputs["v"],
        N_CORES,
    )
    res = run_bass_kernel_spmd(
        nc, in_maps, core_ids=list(range(N_CORES)), trace=trace
    )
    out = np.asarray(res.results[0]["out"], np.float32).reshape(1)
    return out, res


def kernel(**inputs) -> np.ndarray:
    out, _ = run(inputs, trace=False)
    return out



# revision 8
# speedup vs baseline: 1.1530x; 1.1530x over previous
"""Distributed Trainium2 (Bass/Tile) kernel for the DQN-style GNN message-passing
module.

Full-input contract: ``kernel(**inputs)`` takes the unsharded inputs exactly as
produced by ``setup_inputs()`` and returns the full output (shape ``(1,)``).

Strategy (v2 — single collective, replicated iterations):
  - Phase 1 (memory-bound): graph [N, N] row-sharded across 8 cores; each core
    streams its [N/8, N] shard once and computes two per-row scalars:
    s_abs[r] = sum_j |g[r, j]| and s_sum[r] = sum_j g[r, j] (split into column
    halves so the trailing compute after the last DMA is short).
  - Exact relu identity (as the reference): the per-row constant term is
    c[r] = s_abs[r] * A + s_sum[r] * B,  A = 0.5 * |t4| @ theta3,
    B = 0.5 * t4 @ theta3;  a[r] = xf[r] * theta1.
  - ONE AllGather of the per-row scalars ([4, N/8] per core -> [4, N] total,
    ~64 KB) replaces the baseline's 4 serial AllReduces: afterwards EVERY core
    redundantly runs all T iterations over the full N rows, so the global sum
    S_t = u_t.sum(0) is a local row-sum and no further communication is needed.
  - Iteration state US [69, N]: rows 0..63 = u^T (updated in place), row 64 =
    xf (full), rows 65..68 = [sabs_h0; sabs_h1; ssum_h0; ssum_h1] (gathered).
    With host-built stationary M = [-theta2; theta1; A; A; B; B], one matmul
    chain computes pre^T = M^T @ US = (a + c - u @ theta2)^T and
    u' = Relu(pre + (S_t @ theta2)^T) via activation with per-partition bias.
    Matmuls run in float32r (1 cycle/row vs 4 for plain f32); relu+row-sum
    chunks alternate between the Act and DVE engines.
  - Readout is fully local (every core has all of u_T): u_T[v] is a column
    slice (v is baked into the compiled program; programs are cached per v).
"""

from contextlib import ExitStack

import numpy as np

import concourse.bass as bass
import concourse.tile as tile
from concourse import bacc, mybir
from concourse.bass_utils import run_bass_kernel_spmd

F32 = mybir.dt.float32
F32R = mybir.dt.float32r
BF16 = mybir.dt.bfloat16
AX = mybir.AxisListType
ALU = mybir.AluOpType
ACTF = mybir.ActivationFunctionType

N_CORES = 8
DIM = 64

_program_cache: dict = {}


def build_program(R: int, C: int, D: int, T: int, V: int, n_cores: int = N_CORES):
    """Build + compile the per-core SPMD Bass program.

    R: local rows (N / n_cores), C: row length (N), D: dim, T: iterations,
    V: readout row index (baked in).
    """
    assert R % 128 == 0 and D == 64 and C % 1024 == 0
    NB = R // 128  # row-blocks per core
    HC = C // 2  # column half
    KM = D + 6  # contraction rows: u(64) + ones + xf + sabs_h0/h1 + ssum_h0/h1
    FB = 512  # matmul free-dim chunk (one PSUM bank of f32)
    NCHUNK = C // FB

    nc = bacc.Bacc(
        "TRN2",
        target_bir_lowering=False,
        debug=False,
        enable_asserts=True,
        num_devices=n_cores,
    )
    g_d = nc.dram_tensor("g", [R, C], F32, kind="ExternalInput")
    xfo_d = nc.dram_tensor("xfo", [2, C], F32, kind="ExternalInput")
    mneg_d = nc.dram_tensor("mneg", [KM, D], F32, kind="ExternalInput")
    t2_d = nc.dram_tensor("t2", [D, D], F32, kind="ExternalInput")
    t6_d = nc.dram_tensor("t6", [D, D], F32, kind="ExternalInput")
    t7_d = nc.dram_tensor("t7", [D, D], F32, kind="ExternalInput")
    t5c_d = nc.dram_tensor("t5c", [2 * D, 1], F32, kind="ExternalInput")
    out_d = nc.dram_tensor("out", [1, 1], F32, kind="ExternalOutput")
    dbg_d = nc.dram_tensor("dbg", [D, 20], F32, kind="ExternalOutput")
    ident_d = nc.inline_tensor(np.eye(128, dtype=np.float32), name="ident")

    rg = [list(range(n_cores))]

    with ExitStack() as ctx:
        tc = ctx.enter_context(tile.TileContext(nc))
        const = ctx.enter_context(tc.tile_pool(name="const", bufs=1))
        small = ctx.enter_context(tc.tile_pool(name="small", bufs=1))
        gp = ctx.enter_context(tc.tile_pool(name="gp", bufs=3))
        stp = ctx.enter_context(tc.tile_pool(name="stp", bufs=2))
        slp = ctx.enter_context(tc.tile_pool(name="sl", bufs=2))
        dram = ctx.enter_context(tc.tile_pool(name="dram", bufs=1, space="DRAM"))

        # ---- warm-up collective (pays CC init + aligns cores under the DMA)
        dwi = dram.tile([D, 1], F32, tag="dwi")
        dwo = dram.tile([D * n_cores, 1], F32, tag="dwo")
        nc.gpsimd.collective_compute(
            "AllGather",
            ALU.bypass,
            replica_groups=rg,
            ins=[dwi[:].opt()],
            outs=[dwo[:].opt()],
        )

        # ---- constants / persistent tiles
        ident = const.tile([128, 128], F32)
        nc.scalar.dma_start(out=ident[:], in_=ident_d.ap())
        mneg = const.tile([KM, D], F32R)
        nc.scalar.dma_start(out=mneg[:], in_=mneg_d.ap().bitcast(F32R))
        t2 = const.tile([D, D], F32)
        nc.scalar.dma_start(out=t2[:], in_=t2_d.ap())
        t6 = const.tile([D, D], F32)
        nc.scalar.dma_start(out=t6[:], in_=t6_d.ap())
        t7 = const.tile([D, D], F32)
        nc.scalar.dma_start(out=t7[:], in_=t7_d.ap())
        t5c = const.tile([2 * D, 1], F32)
        nc.scalar.dma_start(out=t5c[:], in_=t5c_d.ap())

        # iteration state:
        # [u (64 rows); ones; xf; sabs_h0; sabs_h1; ssum_h0; ssum_h1]
        # (ones at partition 64 pairs with M's device-updated z_t row, which
        # must sit at a 32-aligned partition for the Act-engine copy)
        US = small.tile([KM, C], F32R)
        nc.scalar.dma_start(out=US[D : D + 2, :], in_=xfo_d.ap().bitcast(F32R))

        # per-row partial sums, one column per (half, block):
        # cols 0..NB-1 = sabs_h0, NB..2NB-1 = sabs_h1, then ssum_h0, ssum_h1
        SPA = small.tile([128, 4 * NB], F32)

        # ---- phase 1: stream the graph shard, accumulate per-row sums
        for b in range(NB):
            gt = gp.tile([128, C], F32, tag="gt")
            r0 = b * 128
            nc.sync.dma_start(out=gt[:, 0:HC], in_=g_d.ap()[r0 : r0 + 128, 0:HC])
            nc.sync.dma_start(out=gt[:, HC:C], in_=g_d.ap()[r0 : r0 + 128, HC:C])
            for h in range(2):
                sl = slice(h * HC, (h + 1) * HC)
                st = stp.tile([128, HC], BF16, tag="st")
                ca = (h * NB) + b
                cs = (2 + h) * NB + b
                nc.scalar.activation(
                    out=st[:],
                    in_=gt[:, sl],
                    func=ACTF.Abs,
                    accum_out=SPA[:, ca : ca + 1],
                )
                nc.vector.reduce_sum(
                    out=SPA[:, cs : cs + 1], in_=gt[:, sl], axis=AX.X
                )

        # ---- transpose sums to rows, AllGather, scatter back into US
        with tc.tile_pool(name="psT", bufs=1, space="PSUM") as psT:
            tb = psT.tile([4 * NB, 128], F32, tag="tb")
            nc.tensor.transpose(out=tb[:], in_=SPA[:], identity=ident[:])
            tbs = small.tile([4 * NB, 128], F32)
            nc.vector.tensor_copy(tbs[:], tb[:])
        cin = dram.tile([4 * NB, 128], F32, tag="cin")
        nc.scalar.dma_start(out=cin[:], in_=tbs[:])
        cout = dram.tile([n_cores * 4 * NB, 128], F32, tag="cout")
        nc.gpsimd.collective_compute(
            "AllGather",
            ALU.bypass,
            replica_groups=rg,
            ins=[cin[:].opt()],
            outs=[cout[:].opt()],
        )
        # cout row (i * 4NB + w * NB + b) col c  ->  US row D+1+w, col i*R + b*128 + c
        rap = cout[:].rearrange("(i w b) c -> w i b c", i=n_cores, w=4, b=NB)
        for w in range(4):
            eng = nc.sync if w % 2 == 0 else nc.scalar
            eng.dma_start(out=US[D + 2 + w : D + 3 + w, :], in_=rap[w].bitcast(F32R))

        # ---- T iterations, all rows local (no more collectives)
        # The per-iteration bias z_t = theta2^T @ S_t rides as row KM-1 of the
        # stationary M (against the constant ones row of US), so every relu
        # chunk is bias-free and alternates between the Act and DVE engines.
        psPRE = ctx.enter_context(tc.tile_pool(name="psPRE", bufs=4, space="PSUM"))
        psZ = ctx.enter_context(tc.tile_pool(name="psZ", bufs=2, space="PSUM"))
        Sp = small.tile([D, NCHUNK], F32)
        for t in range(T):
            if t > 0:
                Ssum = slp.tile([D, 1], F32, tag="ss")
                nc.vector.reduce_sum(out=Ssum[:], in_=Sp[:], axis=AX.X)
                zrow = psZ.tile([1, D], F32, tag="zr")
                # z^T = S^T @ theta2 as a row vector, written into M's last row
                nc.tensor.matmul(
                    zrow[:], lhsT=Ssum[:], rhs=t2[:], start=True, stop=True
                )
                nc.scalar.copy(mneg[D : D + 1, :], zrow[:])
            k0 = 0 if t > 0 else D  # first iteration contracts only scalar rows
            for ci in range(NCHUNK):
                h0, h1 = ci * FB, (ci + 1) * FB
                pre = psPRE.tile([D, FB], F32, tag="pre")
                nc.tensor.matmul(
                    pre[:],
                    lhsT=mneg[k0:KM, :],
                    rhs=US[k0:KM, h0:h1],
                    start=True,
                    stop=True,
                )
                if ci % 2 == 0 or ci == NCHUNK - 1:
                    nc.scalar.activation(
                        out=US[0:D, h0:h1],
                        in_=pre[:],
                        func=ACTF.Relu,
                        accum_out=Sp[:, ci : ci + 1],
                    )
                else:
                    nc.vector.tensor_scalar(
                        out=US[0:D, h0:h1],
                        in0=pre[:],
                        scalar1=0.0,
                        scalar2=0.0,
                        op0=ALU.max,
                        op1=ALU.add,
                        accum_out=Sp[:, ci : ci + 1],
                    )

        # ---- final readout (fully local; only core 0's output is consumed)
        with tc.tile_pool(name="psF", bufs=1, space="PSUM") as psF:
            SW = small.tile([D, 2], F32)
            nc.vector.reduce_sum(out=SW[:, 0:1], in_=Sp[:], axis=AX.X)
            nc.scalar.copy(SW[:, 1:2], US[0:D, V : V + 1].bitcast(F32))
            q = psF.tile([2 * D, 1], F32, tag="q")
            nc.tensor.matmul(
                q[0:D, :], lhsT=t6[:], rhs=SW[:, 0:1], start=True, stop=True
            )
            nc.tensor.matmul(
                q[D : 2 * D, :], lhsT=t7[:], rhs=SW[:, 1:2], start=True, stop=True
            )
            rq = small.tile([2 * D, 1], F32)
            nc.scalar.activation(out=rq[:], in_=q[:], func=ACTF.Relu)
            res = psF.tile([1, 1], F32, tag="res")
            nc.tensor.matmul(res[:], lhsT=rq[:], rhs=t5c[:], start=True, stop=True)
            ress = small.tile([1, 1], F32)
            nc.scalar.copy(ress[:], res[:])
            nc.scalar.dma_start(out=out_d.ap(), in_=ress[:])
            dbgt = small.tile([D, 20], F32)
            nc.vector.tensor_copy(dbgt[:, 0:NCHUNK], Sp[:])
            nc.vector.tensor_copy(dbgt[:, NCHUNK:NCHUNK+2], SW[:])
            nc.vector.tensor_copy(dbgt[:, 18:19], US[0:D, V:V+1].bitcast(F32))
            nc.vector.tensor_copy(dbgt[:, 19:20], US[0:D, 0:1].bitcast(F32))
            nc.scalar.dma_start(out=dbg_d.ap(), in_=dbgt[:])

    nc.compile()
    return nc


def get_program(R: int, C: int, D: int, T: int, V: int, n_cores: int = N_CORES):
    key = (R, C, D, T, V, n_cores)
    if key not in _program_cache:
        _program_cache[key] = build_program(R, C, D, T, V, n_cores)
    return _program_cache[key]


def make_in_maps(graph, x, theta1, theta2, theta3, theta4, theta5, theta6, theta7,
                 n_cores: int = N_CORES):
    """Host-side sharding + tiny theta preprocessing."""
    N = graph.shape[0]
    D = theta1.shape[1]
    R = N // n_cores
    f32 = np.float32

    t4 = np.asarray(theta4, f32)[0]
    t3 = np.asarray(theta3, f32)
    A = 0.5 * (np.abs(t4) @ t3)
    B = 0.5 * (t4 @ t3)
    t2 = np.ascontiguousarray(np.asarray(theta2, f32))
    mneg = np.ascontiguousarray(
        np.concatenate(
            [-t2, np.zeros((1, D), f32), np.asarray(theta1, f32),
             A[None, :], A[None, :], B[None, :], B[None, :]],
            axis=0,
        ).astype(f32)
    )  # (D + 6, D); row D (vs US's ones row) is overwritten with z_t on device
    t5c = np.ascontiguousarray(np.asarray(theta5, f32).reshape(2 * D, 1))
    t6 = np.ascontiguousarray(np.asarray(theta6, f32))
    t7 = np.ascontiguousarray(np.asarray(theta7, f32))

    xfo = np.ascontiguousarray(
        np.stack([np.ones(N, f32), np.asarray(x).astype(f32)], axis=0)
    )
    gf = np.asarray(graph, f32)

    in_maps = []
    for i in range(n_cores):
        sl = slice(i * R, (i + 1) * R)
        in_maps.append(
            {
                "g": np.ascontiguousarray(gf[sl]),
                "xfo": xfo,
                "mneg": mneg,
                "t2": t2,
                "t6": t6,
                "t7": t7,
                "t5c": t5c,
            }
        )
    return in_maps


def run(inputs: dict, trace: bool = False):
    """Run the distributed kernel on hardware; returns (output, BassKernelResults)."""
    graph = np.asarray(inputs["graph"])
    N = graph.shape[0]
    D = inputs["theta1"].shape[1]
    T = int(inputs["T"])
    V = int(inputs["v"])
    R = N // N_CORES

    nc = get_program(R, N, D, T, V, N_CORES)
    in_maps = make_in_maps(
        graph,
        inputs["x"],
        inputs["theta1"],
        inputs["theta2"],
        inputs["theta3"],
        inputs["theta4"],
        inputs["theta5"],
        inputs["theta6"],
        inputs["theta7"],
        N_CORES,
    )
    res = run_bass_kernel_spmd(
        nc, in_maps, core_ids=list(range(N_CORES)), trace=trace
    )
    out = np.asarray(res.results[0]["out"], np.float32).reshape(1)
    return out, res


def kernel(**inputs) -> np.ndarray:
    out, _ = run(inputs, trace=False)
    return out


# revision 9
# speedup vs baseline: 1.2386x; 1.0743x over previous
"""Distributed Trainium2 (Bass/Tile) kernel for the DQN-style GNN message-passing
module.

Full-input contract: ``kernel(**inputs)`` takes the unsharded inputs exactly as
produced by ``setup_inputs()`` and returns the full output (shape ``(1,)``).

Strategy (v2 — single collective, replicated iterations):
  - Phase 1 (memory-bound): graph [N, N] row-sharded across 8 cores; each core
    streams its [N/8, N] shard once and computes two per-row scalars:
    s_abs[r] = sum_j |g[r, j]| and s_sum[r] = sum_j g[r, j] (split into column
    halves so the trailing compute after the last DMA is short).
  - Exact relu identity (as the reference): the per-row constant term is
    c[r] = s_abs[r] * A + s_sum[r] * B,  A = 0.5 * |t4| @ theta3,
    B = 0.5 * t4 @ theta3;  a[r] = xf[r] * theta1.
  - ONE AllGather of the per-row scalars ([4, N/8] per core -> [4, N] total,
    ~64 KB) replaces the baseline's 4 serial AllReduces: afterwards EVERY core
    redundantly runs all T iterations over the full N rows, so the global sum
    S_t = u_t.sum(0) is a local row-sum and no further communication is needed.
  - Iteration state US [69, N]: rows 0..63 = u^T (updated in place), row 64 =
    xf (full), rows 65..68 = [sabs_h0; sabs_h1; ssum_h0; ssum_h1] (gathered).
    With host-built stationary M = [-theta2; theta1; A; A; B; B], one matmul
    chain computes pre^T = M^T @ US = (a + c - u @ theta2)^T and
    u' = Relu(pre + (S_t @ theta2)^T) via activation with per-partition bias.
    Matmuls run in float32r (1 cycle/row vs 4 for plain f32); relu+row-sum
    chunks alternate between the Act and DVE engines.
  - Readout is fully local (every core has all of u_T): u_T[v] is a column
    slice (v is baked into the compiled program; programs are cached per v).
"""

from contextlib import ExitStack

import numpy as np

import concourse.bass as bass
import concourse.tile as tile
from concourse import bacc, mybir
from concourse.bass_utils import run_bass_kernel_spmd

F32 = mybir.dt.float32
F32R = mybir.dt.float32r
BF16 = mybir.dt.bfloat16
AX = mybir.AxisListType
ALU = mybir.AluOpType
ACTF = mybir.ActivationFunctionType

N_CORES = 8
DIM = 64

_program_cache: dict = {}


def build_program(R: int, C: int, D: int, T: int, V: int, n_cores: int = N_CORES):
    """Build + compile the per-core SPMD Bass program.

    R: local rows (N / n_cores), C: row length (N), D: dim, T: iterations,
    V: readout row index (baked in).
    """
    assert R % 128 == 0 and D == 64 and C % 1024 == 0
    NB = R // 128  # row-blocks per core
    HC = C // 2  # column half
    KM = D + 6  # contraction rows: u(64) + ones + xf + sabs_h0/h1 + ssum_h0/h1
    FB = 512  # matmul free-dim chunk (one PSUM bank of f32)
    NCHUNK = C // FB

    nc = bacc.Bacc(
        "TRN2",
        target_bir_lowering=False,
        debug=False,
        enable_asserts=True,
        num_devices=n_cores,
    )
    g_d = nc.dram_tensor("g", [R, C], F32, kind="ExternalInput")
    xfo_d = nc.dram_tensor("xfo", [2, C], F32, kind="ExternalInput")
    mneg_d = nc.dram_tensor("mneg", [KM, D], F32, kind="ExternalInput")
    t2_d = nc.dram_tensor("t2", [D, D], F32, kind="ExternalInput")
    t6_d = nc.dram_tensor("t6", [D, D], F32, kind="ExternalInput")
    t7_d = nc.dram_tensor("t7", [D, D], F32, kind="ExternalInput")
    t5c_d = nc.dram_tensor("t5c", [2 * D, 1], F32, kind="ExternalInput")
    out_d = nc.dram_tensor("out", [1, 1], F32, kind="ExternalOutput")
    ident_d = nc.inline_tensor(np.eye(128, dtype=np.float32), name="ident")

    rg = [list(range(n_cores))]

    with ExitStack() as ctx:
        tc = ctx.enter_context(tile.TileContext(nc))
        const = ctx.enter_context(tc.tile_pool(name="const", bufs=1))
        small = ctx.enter_context(tc.tile_pool(name="small", bufs=1))
        gp = ctx.enter_context(tc.tile_pool(name="gp", bufs=3))
        stp = ctx.enter_context(tc.tile_pool(name="stp", bufs=2))
        slp = ctx.enter_context(tc.tile_pool(name="sl", bufs=2))
        dram = ctx.enter_context(tc.tile_pool(name="dram", bufs=1, space="DRAM"))

        # ---- warm-up collective (pays CC init + aligns cores under the DMA)
        dwi = dram.tile([D, 1], F32, tag="dwi")
        dwo = dram.tile([D * n_cores, 1], F32, tag="dwo")
        nc.gpsimd.collective_compute(
            "AllGather",
            ALU.bypass,
            replica_groups=rg,
            ins=[dwi[:].opt()],
            outs=[dwo[:].opt()],
        )

        # ---- constants / persistent tiles
        ident = const.tile([128, 128], F32)
        nc.scalar.dma_start(out=ident[:], in_=ident_d.ap())
        mneg = const.tile([KM, D], F32R)
        nc.scalar.dma_start(out=mneg[:], in_=mneg_d.ap().bitcast(F32R))
        t2 = const.tile([D, D], F32)
        nc.scalar.dma_start(out=t2[:], in_=t2_d.ap())
        t6 = const.tile([D, D], F32)
        nc.scalar.dma_start(out=t6[:], in_=t6_d.ap())
        t7 = const.tile([D, D], F32)
        nc.scalar.dma_start(out=t7[:], in_=t7_d.ap())
        t5c = const.tile([2 * D, 1], F32)
        nc.scalar.dma_start(out=t5c[:], in_=t5c_d.ap())

        # iteration state:
        # [u (64 rows); ones; xf; sabs_h0; sabs_h1; ssum_h0; ssum_h1]
        # (ones at partition 64 pairs with M's device-updated z_t row, which
        # must sit at a 32-aligned partition for the Act-engine copy)
        US = small.tile([KM, C], F32R)
        nc.scalar.dma_start(out=US[D : D + 2, :], in_=xfo_d.ap().bitcast(F32R))

        # per-row partial sums, one column per (half, block):
        # cols 0..NB-1 = sabs_h0, NB..2NB-1 = sabs_h1, then ssum_h0, ssum_h1
        SPA = small.tile([128, 4 * NB], F32)

        # ---- phase 1: stream the graph shard, accumulate per-row sums
        for b in range(NB):
            gt = gp.tile([128, C], F32, tag="gt")
            r0 = b * 128
            nc.sync.dma_start(out=gt[:, 0:HC], in_=g_d.ap()[r0 : r0 + 128, 0:HC])
            nc.sync.dma_start(out=gt[:, HC:C], in_=g_d.ap()[r0 : r0 + 128, HC:C])
            for h in range(2):
                sl = slice(h * HC, (h + 1) * HC)
                st = stp.tile([128, HC], BF16, tag="st")
                ca = (h * NB) + b
                cs = (2 + h) * NB + b
                nc.scalar.activation(
                    out=st[:],
                    in_=gt[:, sl],
                    func=ACTF.Abs,
                    accum_out=SPA[:, ca : ca + 1],
                )
                nc.vector.reduce_sum(
                    out=SPA[:, cs : cs + 1], in_=gt[:, sl], axis=AX.X
                )

        # ---- transpose sums to rows, AllGather, scatter back into US
        with tc.tile_pool(name="psT", bufs=1, space="PSUM") as psT:
            tb = psT.tile([4 * NB, 128], F32, tag="tb")
            nc.tensor.transpose(out=tb[:], in_=SPA[:], identity=ident[:])
            tbs = small.tile([4 * NB, 128], F32)
            nc.vector.tensor_copy(tbs[:], tb[:])
        cin = dram.tile([4 * NB, 128], F32, tag="cin")
        nc.scalar.dma_start(out=cin[:], in_=tbs[:])
        cout = dram.tile([n_cores * 4 * NB, 128], F32, tag="cout")
        nc.gpsimd.collective_compute(
            "AllGather",
            ALU.bypass,
            replica_groups=rg,
            ins=[cin[:].opt()],
            outs=[cout[:].opt()],
        )
        # cout row (i * 4NB + w * NB + b) col c  ->  US row D+1+w, col i*R + b*128 + c
        rap = cout[:].rearrange("(i w b) c -> w i b c", i=n_cores, w=4, b=NB)
        for w in range(4):
            eng = nc.sync if w % 2 == 0 else nc.scalar
            eng.dma_start(out=US[D + 2 + w : D + 3 + w, :], in_=rap[w].bitcast(F32R))

        # ---- T iterations, all rows local (no more collectives)
        # The per-iteration bias z_t = theta2^T @ S_t rides as row KM-1 of the
        # stationary M (against the constant ones row of US), so every relu
        # chunk is bias-free and alternates between the Act and DVE engines.
        psPRE = ctx.enter_context(tc.tile_pool(name="psPRE", bufs=4, space="PSUM"))
        psZ = ctx.enter_context(tc.tile_pool(name="psZ", bufs=2, space="PSUM"))
        Sp = small.tile([D, NCHUNK], F32)
        for t in range(T):
            if t > 0:
                Ssum = slp.tile([D, 1], F32, tag="ss")
                nc.vector.reduce_sum(out=Ssum[:], in_=Sp[:], axis=AX.X)
                zrow = psZ.tile([1, D], F32, tag="zr")
                # z^T = S^T @ theta2 as a row vector, written into M's last row
                nc.tensor.matmul(
                    zrow[:], lhsT=Ssum[:], rhs=t2[:], start=True, stop=True
                )
                nc.scalar.copy(mneg[D : D + 1, :], zrow[:])
            k0 = 0 if t > 0 else D  # first iteration contracts only scalar rows
            for ci in range(NCHUNK):
                h0, h1 = ci * FB, (ci + 1) * FB
                pre = psPRE.tile([D, FB], F32, tag="pre")
                nc.tensor.matmul(
                    pre[:],
                    lhsT=mneg[k0:KM, :],
                    rhs=US[k0:KM, h0:h1],
                    start=True,
                    stop=True,
                )
                if ci % 2 == 0 or ci == NCHUNK - 1:
                    nc.scalar.activation(
                        out=US[0:D, h0:h1],
                        in_=pre[:],
                        func=ACTF.Relu,
                        accum_out=Sp[:, ci : ci + 1],
                    )
                else:
                    nc.vector.tensor_scalar(
                        out=US[0:D, h0:h1],
                        in0=pre[:],
                        scalar1=0.0,
                        scalar2=0.0,
                        op0=ALU.max,
                        op1=ALU.add,
                        accum_out=Sp[:, ci : ci + 1],
                    )

        # ---- final readout (fully local; only core 0's output is consumed)
        with tc.tile_pool(name="psF", bufs=1, space="PSUM") as psF:
            SW = small.tile([D, 2], F32)
            nc.vector.reduce_sum(out=SW[:, 0:1], in_=Sp[:], axis=AX.X)
            nc.scalar.copy(SW[:, 1:2], US[0:D, V : V + 1].bitcast(F32))
            q = psF.tile([2 * D, 1], F32, tag="q")
            nc.tensor.matmul(
                q[0:D, :], lhsT=t6[:], rhs=SW[:, 0:1], start=True, stop=True
            )
            nc.tensor.matmul(
                q[D : 2 * D, :], lhsT=t7[:], rhs=SW[:, 1:2], start=True, stop=True
            )
            rq = small.tile([2 * D, 1], F32)
            nc.scalar.activation(out=rq[:], in_=q[:], func=ACTF.Relu)
            res = psF.tile([1, 1], F32, tag="res")
            nc.tensor.matmul(res[:], lhsT=rq[:], rhs=t5c[:], start=True, stop=True)
            ress = small.tile([1, 1], F32)
            nc.scalar.copy(ress[:], res[:])
            nc.scalar.dma_start(out=out_d.ap(), in_=ress[:])

    nc.compile()
    return nc


def get_program(R: int, C: int, D: int, T: int, V: int, n_cores: int = N_CORES):
    key = (R, C, D, T, V, n_cores)
    if key not in _program_cache:
        _program_cache[key] = build_program(R, C, D, T, V, n_cores)
    return _program_cache[key]


def make_in_maps(graph, x, theta1, theta2, theta3, theta4, theta5, theta6, theta7,
                 n_cores: int = N_CORES):
    """Host-side sharding + tiny theta preprocessing."""
    N = graph.shape[0]
    D = theta1.shape[1]
    R = N // n_cores
    f32 = np.float32

    t4 = np.asarray(theta4, f32)[0]
    t3 = np.asarray(theta3, f32)
    A = 0.5 * (np.abs(t4) @ t3)
    B = 0.5 * (t4 @ t3)
    t2 = np.ascontiguousarray(np.asarray(theta2, f32))
    mneg = np.ascontiguousarray(
        np.concatenate(
            [-t2, np.zeros((1, D), f32), np.asarray(theta1, f32),
             A[None, :], A[None, :], B[None, :], B[None, :]],
            axis=0,
        ).astype(f32)
    )  # (D + 6, D); row D (vs US's ones row) is overwritten with z_t on device
    t5c = np.ascontiguousarray(np.asarray(theta5, f32).reshape(2 * D, 1))
    t6 = np.ascontiguousarray(np.asarray(theta6, f32))
    t7 = np.ascontiguousarray(np.asarray(theta7, f32))

    xfo = np.ascontiguousarray(
        np.stack([np.ones(N, f32), np.asarray(x).astype(f32)], axis=0)
    )
    gf = np.asarray(graph, f32)

    in_maps = []
    for i in range(n_cores):
        sl = slice(i * R, (i + 1) * R)
        in_maps.append(
            {
                "g": np.ascontiguousarray(gf[sl]),
                "xfo": xfo,
                "mneg": mneg,
                "t2": t2,
                "t6": t6,
                "t7": t7,
                "t5c": t5c,
            }
        )
    return in_maps


def run(inputs: dict, trace: bool = False):
    """Run the distributed kernel on hardware; returns (output, BassKernelResults)."""
    graph = np.asarray(inputs["graph"])
    N = graph.shape[0]
    D = inputs["theta1"].shape[1]
    T = int(inputs["T"])
    V = int(inputs["v"])
    R = N // N_CORES

    nc = get_program(R, N, D, T, V, N_CORES)
    in_maps = make_in_maps(
        graph,
        inputs["x"],
        inputs["theta1"],
        inputs["theta2"],
        inputs["theta3"],
        inputs["theta4"],
        inputs["theta5"],
        inputs["theta6"],
        inputs["theta7"],
        N_CORES,
    )
    res = run_bass_kernel_spmd(
        nc, in_maps, core_ids=list(range(N_CORES)), trace=trace
    )
    out = np.asarray(res.results[0]["out"], np.float32).reshape(1)
    return out, res


def kernel(**inputs) -> np.ndarray:
    out, _ = run(inputs, trace=False)
    return out


# revision 11
# speedup vs baseline: 1.2908x; 1.0421x over previous
"""Distributed Trainium2 (Bass/Tile) kernel for the DQN-style GNN message-passing
module.

Full-input contract: ``kernel(**inputs)`` takes the unsharded inputs exactly as
produced by ``setup_inputs()`` and returns the full output (shape ``(1,)``).

Strategy (v3 — two-stage AllGather, replicated f32r iterations):
  - Phase 1 (memory-bound): graph [N, N] row-sharded across 8 cores; each core
    streams its [N/8, N] shard once (column halves, two DMA queues) and
    accumulates two per-row scalars: s_abs[r] = sum_j |g[r, j]| (Act engine,
    Abs+accum) and s_sum[r] = sum_j g[r, j] (DVE reduce).
  - Exact relu identity (as the reference): the per-row constant term is
    c[r] = s_abs[r] * A + s_sum[r] * B,  A = 0.5 * |t4| @ theta3,
    B = 0.5 * t4 @ theta3;  a[r] = xf[r] * theta1.
  - The per-row scalars are AllGathered in TWO stages: blocks 0..6 as soon as
    they are done (hidden under the last block's DMA) and block 7 at the end
    (only its ~7us latency is exposed). Afterwards EVERY core redundantly runs
    all T iterations over the full N rows, so S_t = u_t.sum(0) is a local
    row-sum — no per-iteration collectives at all. The t=0 pass uses 512-col
    chunks ordered so that chunks free of block-7 columns run under the
    second AllGather's shadow.
  - Iteration state US [70, N] (float32r: 1 cycle/row matmuls vs 4 for f32):
    rows 0..63 = u^T (updated in place), 64 = ones, 65 = xf, 66..69 =
    [sabs_h0; sabs_h1; ssum_h0; ssum_h1]. The per-iteration bias
    z_t = theta2^T S_t rides as row 64 of the host-built stationary
    M = [-theta2; z; theta1; A; A; B; B] (z written on device against the
    ones row), so pre^T = M^T @ US in one matmul chain and every relu chunk
    is bias-free: u' = relu(pre), with per-chunk row-sum accumulators
    alternating between the Act and DVE engines ([64, 1024] chunks).
  - Readout is fully local (every core has all of u_T): u_T[v] is a column
    slice (v is baked into the compiled program; programs are cached per v).
"""

from contextlib import ExitStack

import numpy as np

import concourse.bass as bass
import concourse.tile as tile
from concourse import bacc, mybir
from concourse.bass_utils import run_bass_kernel_spmd

F32 = mybir.dt.float32
F32R = mybir.dt.float32r
BF16 = mybir.dt.bfloat16
AX = mybir.AxisListType
ALU = mybir.AluOpType
ACTF = mybir.ActivationFunctionType

N_CORES = 8
DIM = 64

_program_cache: dict = {}


def build_program(R: int, C: int, D: int, T: int, V: int, n_cores: int = N_CORES):
    """Build + compile the per-core SPMD Bass program.

    R: local rows (N / n_cores), C: row length (N), D: dim, T: iterations,
    V: readout row index (baked in).
    """
    assert R % 128 == 0 and D == 64 and C % 1024 == 0
    NB = R // 128  # row-blocks per core
    NBA = NB - 1  # blocks in the early AllGather group
    HC = C // 2  # column half
    KM = D + 6  # contraction rows: u(64) + ones + xf + sabs_h0/h1 + ssum_h0/h1
    FB = 512  # matmul free-dim chunk (one PSUM bank of f32)
    AB = 1024  # act/relu chunk (two PSUM banks)
    NCH_MM = C // FB
    NCH_ACT = C // AB

    nc = bacc.Bacc(
        "TRN2",
        target_bir_lowering=False,
        debug=False,
        enable_asserts=True,
        num_devices=n_cores,
    )
    g_d = nc.dram_tensor("g", [R, C], F32, kind="ExternalInput")
    xfo_d = nc.dram_tensor("xfo", [2, C], F32, kind="ExternalInput")
    mneg_d = nc.dram_tensor("mneg", [KM, D], F32, kind="ExternalInput")
    t2_d = nc.dram_tensor("t2", [D, D], F32, kind="ExternalInput")
    t6_d = nc.dram_tensor("t6", [D, D], F32, kind="ExternalInput")
    t7_d = nc.dram_tensor("t7", [D, D], F32, kind="ExternalInput")
    t5c_d = nc.dram_tensor("t5c", [2 * D, 1], F32, kind="ExternalInput")
    out_d = nc.dram_tensor("out", [1, 1], F32, kind="ExternalOutput")
    ident_d = nc.inline_tensor(np.eye(128, dtype=np.float32), name="ident")

    rg = [list(range(n_cores))]

    with ExitStack() as ctx:
        tc = ctx.enter_context(tile.TileContext(nc))
        const = ctx.enter_context(tc.tile_pool(name="const", bufs=1))
        small = ctx.enter_context(tc.tile_pool(name="small", bufs=1))
        gp = ctx.enter_context(tc.tile_pool(name="gp", bufs=4))
        stp = ctx.enter_context(tc.tile_pool(name="stp", bufs=1))
        slp = ctx.enter_context(tc.tile_pool(name="sl", bufs=2))
        dram = ctx.enter_context(tc.tile_pool(name="dram", bufs=1, space="DRAM"))

        # ---- warm-up collective (pays CC init + aligns cores under the DMA)
        dwi = dram.tile([D, 1], F32, tag="dwi")
        dwo = dram.tile([D * n_cores, 1], F32, tag="dwo")
        nc.gpsimd.collective_compute(
            "AllGather",
            ALU.bypass,
            replica_groups=rg,
            ins=[dwi[:].opt()],
            outs=[dwo[:].opt()],
        )

        # ---- constants / persistent tiles (scalar queue)
        ident = const.tile([128, 128], F32)
        nc.scalar.dma_start(out=ident[:], in_=ident_d.ap())
        mneg = const.tile([KM, D], F32R)
        nc.scalar.dma_start(out=mneg[:], in_=mneg_d.ap().bitcast(F32R))
        t2 = const.tile([D, D], F32)
        nc.scalar.dma_start(out=t2[:], in_=t2_d.ap())
        t6 = const.tile([D, D], F32)
        nc.scalar.dma_start(out=t6[:], in_=t6_d.ap())
        t7 = const.tile([D, D], F32)
        nc.scalar.dma_start(out=t7[:], in_=t7_d.ap())
        t5c = const.tile([2 * D, 1], F32)
        nc.scalar.dma_start(out=t5c[:], in_=t5c_d.ap())

        # iteration state:
        # [u (64 rows); ones; xf; sabs_h0; sabs_h1; ssum_h0; ssum_h1]
        # (ones at partition 64 pairs with M's device-updated z_t row, which
        # must sit at a 32-aligned partition for the engine copy)
        US = small.tile([KM, C], F32R)
        nc.scalar.dma_start(out=US[D : D + 2, :], in_=xfo_d.ap().bitcast(F32R))

        # per-row partial sums, one column per (half, block); group A holds
        # blocks 0..NBA-1, group B the last block
        SPA_A = small.tile([128, 4 * NBA], F32)
        SPA_B = small.tile([128, 4], F32)

        cin_a = dram.tile([4 * NBA, 128], F32, tag="cin_a")
        cout_a = dram.tile([n_cores * 4 * NBA, 128], F32, tag="cout_a")
        cin_b = dram.tile([4, 128], F32, tag="cin_b")
        cout_b = dram.tile([n_cores * 4, 128], F32, tag="cout_b")

        psT_ctx = tc.tile_pool(name="psT", bufs=1, space="PSUM")
        psT = psT_ctx.__enter__()

        # ---- phase 1: stream the graph shard, accumulate per-row sums.
        # Even blocks ride the sync DMA queue, odd blocks the scalar queue.
        for b in range(NB):
            gt = gp.tile([128, C], F32, tag="gt")
            r0 = b * 128
            eng = nc.sync if b % 2 == 0 else nc.scalar
            eng.dma_start(out=gt[:, 0:HC], in_=g_d.ap()[r0 : r0 + 128, 0:HC])
            eng.dma_start(out=gt[:, HC:C], in_=g_d.ap()[r0 : r0 + 128, HC:C])
            for h in range(2):
                sl = slice(h * HC, (h + 1) * HC)
                st = stp.tile([128, HC], BF16, tag="st")
                if b < NBA:
                    spa, ca, cs = SPA_A, h * NBA + b, (2 + h) * NBA + b
                else:
                    spa, ca, cs = SPA_B, h, 2 + h
                nc.scalar.activation(
                    out=st[:],
                    in_=gt[:, sl],
                    func=ACTF.Abs,
                    accum_out=spa[:, ca : ca + 1],
                )
                nc.vector.reduce_sum(
                    out=spa[:, cs : cs + 1], in_=gt[:, sl], axis=AX.X
                )
            if b == NBA - 1:
                # group A is complete: transpose and kick its AllGather while
                # the last block is still streaming
                tbA = psT.tile([4 * NBA, 128], F32, tag="tbA")
                nc.tensor.transpose(out=tbA[:], in_=SPA_A[:], identity=ident[:])
                tbsA = small.tile([4 * NBA, 128], F32)
                nc.vector.tensor_copy(tbsA[:], tbA[:])
                nc.sync.dma_start(out=cin_a[:], in_=tbsA[:])
                nc.gpsimd.collective_compute(
                    "AllGather",
                    ALU.bypass,
                    replica_groups=rg,
                    ins=[cin_a[:].opt()],
                    outs=[cout_a[:].opt()],
                )

        # ---- group B (last block) transpose + AllGather
        tbB = psT.tile([4, 128], F32, tag="tbB")
        nc.tensor.transpose(out=tbB[:], in_=SPA_B[:], identity=ident[:])
        tbsB = small.tile([4, 128], F32)
        nc.vector.tensor_copy(tbsB[:], tbB[:])
        psT_ctx.__exit__(None, None, None)

        # scatter group A's gathered sums into US (sync queue: queued after all
        # graph DMAs, so the wait on the AllGather doesn't block streaming)
        rapA = cout_a[:].rearrange("(i w b) c -> w i b c", i=n_cores, w=4, b=NBA)
        for w in range(4):
            rowap = US[D + 2 + w : D + 3 + w, :].rearrange(
                "p (i q) -> p i q", i=n_cores
            )
            nc.sync.dma_start(
                out=rowap[:, :, 0 : NBA * 128], in_=rapA[w].bitcast(F32R)
            )
        nc.sync.dma_start(out=cin_b[:], in_=tbsB[:])
        nc.gpsimd.collective_compute(
            "AllGather",
            ALU.bypass,
            replica_groups=rg,
            ins=[cin_b[:].opt()],
            outs=[cout_b[:].opt()],
        )
        rapB = cout_b[:].rearrange("(i w b) c -> w i b c", i=n_cores, w=4, b=1)
        for w in range(4):
            rowap = US[D + 2 + w : D + 3 + w, :].rearrange(
                "p (i q) -> p i q", i=n_cores
            )
            nc.sync.dma_start(
                out=rowap[:, :, NBA * 128 : R], in_=rapB[w].bitcast(F32R)
            )

        Sp = small.tile([D, NCH_MM], F32)

        # ---- t=0: u1 = relu(a + c), contracting only the scalar rows.
        # 512-col chunks; chunks without block-7 columns (even ci) depend only
        # on the early AllGather and run under the late one's shadow.
        with tc.tile_pool(name="psP0", bufs=4, space="PSUM") as psP0:
            order0 = [ci for ci in range(NCH_MM) if ci % 2 == 0] + [
                ci for ci in range(NCH_MM) if ci % 2 == 1
            ]
            for j, ci in enumerate(order0):
                h0, h1 = ci * FB, (ci + 1) * FB
                pre = psP0.tile([D, FB], F32, tag="pre0")
                nc.tensor.matmul(
                    pre[:],
                    lhsT=mneg[D:KM, :],
                    rhs=US[D:KM, h0:h1],
                    start=True,
                    stop=True,
                )
                if j % 2 == 0:
                    nc.scalar.activation(
                        out=US[0:D, h0:h1],
                        in_=pre[:],
                        func=ACTF.Relu,
                        accum_out=Sp[:, ci : ci + 1],
                    )
                else:
                    nc.vector.tensor_scalar(
                        out=US[0:D, h0:h1],
                        in0=pre[:],
                        scalar1=0.0,
                        scalar2=0.0,
                        op0=ALU.max,
                        op1=ALU.add,
                        accum_out=Sp[:, ci : ci + 1],
                    )

        # ---- t=1..T-1: full-width iterations, [64, 1024] act chunks
        with tc.tile_pool(name="psPRE", bufs=3, space="PSUM") as psPRE, \
                tc.tile_pool(name="psZ", bufs=1, space="PSUM") as psZ:
            for t in range(1, T):
                ncols = NCH_MM if t == 1 else NCH_ACT
                Ssum = slp.tile([D, 1], F32, tag="ss")
                nc.vector.reduce_sum(out=Ssum[:], in_=Sp[:, 0:ncols], axis=AX.X)
                zrow = psZ.tile([1, D], F32, tag="zr")
                # z^T = S^T @ theta2 as a row vector, into M's z row
                nc.tensor.matmul(
                    zrow[:], lhsT=Ssum[:], rhs=t2[:], start=True, stop=True
                )
                nc.vector.tensor_copy(mneg[D : D + 1, :], zrow[:])
                for ci in range(NCH_ACT):
                    a0, a1 = ci * AB, (ci + 1) * AB
                    pre = psPRE.tile([D, AB], F32, tag="pre")
                    for f in range(AB // FB):
                        nc.tensor.matmul(
                            pre[:, f * FB : (f + 1) * FB],
                            lhsT=mneg[:],
                            rhs=US[:, a0 + f * FB : a0 + (f + 1) * FB],
                            start=True,
                            stop=True,
                        )
                    if ci % 2 == 0:
                        nc.scalar.activation(
                            out=US[0:D, a0:a1],
                            in_=pre[:],
                            func=ACTF.Relu,
                            accum_out=Sp[:, ci : ci + 1],
                        )
                    else:
                        nc.vector.tensor_scalar(
                            out=US[0:D, a0:a1],
                            in0=pre[:],
                            scalar1=0.0,
                            scalar2=0.0,
                            op0=ALU.max,
                            op1=ALU.add,
                            accum_out=Sp[:, ci : ci + 1],
                        )

        # ---- final readout (fully local; only core 0's output is consumed)
        with tc.tile_pool(name="psF", bufs=1, space="PSUM") as psF:
            SW = small.tile([D, 2], F32)
            # u_T[v]: ready as soon as t=T-1's covering chunk lands; gpsimd is
            # idle so this doesn't queue behind the act engines
            nc.gpsimd.tensor_copy(SW[:, 1:2], US[0:D, V : V + 1].bitcast(F32))
            q = psF.tile([2 * D, 1], F32, tag="q")
            nc.tensor.matmul(
                q[D : 2 * D, :], lhsT=t7[:], rhs=SW[:, 1:2], start=True, stop=True
            )
            ncols = NCH_MM if T == 1 else NCH_ACT
            nc.vector.reduce_sum(out=SW[:, 0:1], in_=Sp[:, 0:ncols], axis=AX.X)
            nc.tensor.matmul(
                q[0:D, :], lhsT=t6[:], rhs=SW[:, 0:1], start=True, stop=True
            )
            rq = small.tile([2 * D, 1], F32)
            nc.scalar.activation(out=rq[:], in_=q[:], func=ACTF.Relu)
            res = psF.tile([1, 1], F32, tag="res")
            nc.tensor.matmul(res[:], lhsT=rq[:], rhs=t5c[:], start=True, stop=True)
            ress = small.tile([1, 1], F32)
            nc.scalar.copy(ress[:], res[:])
            nc.scalar.dma_start(out=out_d.ap(), in_=ress[:])

    nc.compile()
    return nc


def get_program(R: int, C: int, D: int, T: int, V: int, n_cores: int = N_CORES):
    key = (R, C, D, T, V, n_cores)
    if key not in _program_cache:
        _program_cache[key] = build_program(R, C, D, T, V, n_cores)
    return _program_cache[key]


def make_in_maps(graph, x, theta1, theta2, theta3, theta4, theta5, theta6, theta7,
                 n_cores: int = N_CORES):
    """Host-side sharding + tiny theta preprocessing."""
    N = graph.shape[0]
    D = theta1.shape[1]
    R = N // n_cores
    f32 = np.float32

    t4 = np.asarray(theta4, f32)[0]
    t3 = np.asarray(theta3, f32)
    A = 0.5 * (np.abs(t4) @ t3)
    B = 0.5 * (t4 @ t3)
    t2 = np.ascontiguousarray(np.asarray(theta2, f32))
    mneg = np.ascontiguousarray(
        np.concatenate(
            [-t2, np.zeros((1, D), f32), np.asarray(theta1, f32),
             A[None, :], A[None, :], B[None, :], B[None, :]],
            axis=0,
        ).astype(f32)
    )  # (D + 6, D); row D (vs US's ones row) is overwritten with z_t on device
    t5c = np.ascontiguousarray(np.asarray(theta5, f32).reshape(2 * D, 1))
    t6 = np.ascontiguousarray(np.asarray(theta6, f32))
    t7 = np.ascontiguousarray(np.asarray(theta7, f32))

    xfo = np.ascontiguousarray(
        np.stack([np.ones(N, f32), np.asarray(x).astype(f32)], axis=0)
    )
    gf = np.asarray(graph, f32)

    in_maps = []
    for i in range(n_cores):
        sl = slice(i * R, (i + 1) * R)
        in_maps.append(
            {
                "g": np.ascontiguousarray(gf[sl]),
                "xfo": xfo,
                "mneg": mneg,
                "t2": t2,
                "t6": t6,
                "t7": t7,
                "t5c": t5c,
            }
        )
    return in_maps


def run(inputs: dict, trace: bool = False):
    """Run the distributed kernel on hardware; returns (output, BassKernelResults)."""
    graph = np.asarray(inputs["graph"])
    N = graph.shape[0]
    D = inputs["theta1"].shape[1]
    T = int(inputs["T"])
    V = int(inputs["v"])
    R = N // N_CORES

    nc = get_program(R, N, D, T, V, N_CORES)
    in_maps = make_in_maps(
        graph,
        inputs["x"],
        inputs["theta1"],
        inputs["theta2"],
        inputs["theta3"],
        inputs["theta4"],
        inputs["theta5"],
        inputs["theta6"],
        inputs["theta7"],
        N_CORES,
    )
    res = run_bass_kernel_spmd(
        nc, in_maps, core_ids=list(range(N_CORES)), trace=trace
    )
    out = np.asarray(res.results[0]["out"], np.float32).reshape(1)
    return out, res


def kernel(**inputs) -> np.ndarray:
    out, _ = run(inputs, trace=False)
    return out
